# revision 48
# baseline (speedup 1.0000x reference)
"""MoE (top-2 of 8 experts, SwiGLU FFN) on 8 Trainium2 NeuronCores.

Strategy (expert-parallel, per the sharding hint):
 - Host: router matmul (f64) + top-2 + softmax gates; dispatch tokens to
   their experts (gather), pad each expert's token list to a uniform
   capacity C.  One expert per core.
 - Device (per core): dense SwiGLU FFN over its C gathered tokens in a
   feature-major (transposed) layout so the per-feature biases are
   per-partition scalars:
       hT = W1[e] @ xgT          (PE, fp16 x fp16 -> fp32 PSUM)
       aT = (h1T+b1a) * silu(h2T+b1b)   (ACT + DVE)
       yT = W2[e] @ aT + b2      (PE, ACT)
 - Host: gather back, apply gate weights, scatter-add into the output.

Shapes are hardcoded for the problem: x [2,2048,1024], E=8, K=2, D=1024,
F=2048.
"""

import os

import numpy as np

import concourse.bacc as bacc
import concourse.tile as tile
from concourse import mybir
from concourse.bass_utils import run_bass_kernel_spmd

B, S, D = 2, 2048, 1024
T = B * S
E = 8
K = 2
F = 2048
TWOF = 2 * F
KT_D = D // 128    # 8 contraction tiles for matmul 1
KT_F = F // 128    # 16 contraction tiles for matmul 2
NF1 = TWOF // 128  # 32 output feature chunks of matmul 1
NF2 = D // 128     # 8 output feature chunks of matmul 2
NT = 512           # token tile (matmul moving free dim)
# w1 chunk storage order (queue-serving): gpsimd slots 0-8, sync 9-15
W1_ORDER = [0, 1, 3, 5, 7, 9, 11, 13, 15, 2, 4, 6, 8, 10, 12, 14]
W1_SLOT = {i: s for s, i in enumerate(W1_ORDER)}

_NC_CACHE = {}
_W_CACHE = {}


def _token_tiles(C):
    """First tile 384: small enough that its xg DMA (the head's serial
    critical path, ~2KB/token) lands early, large enough that per-chunk
    weight consumption (16 matmuls/chunk) stays under the w1 DMA supply
    rate.  Middle tiles 512; last tile in [240, 512] (>=240 keeps the
    128-cycle LDWEIGHTS hidden under the previous matmul), small-ish so
    the post-last-matmul drain (ACT + output DMA) is short."""
    szs = [min(384, C)]
    rem = C - szs[0]
    if 0 < rem < 240:
        take = 240 - rem
        szs[0] -= take
        rem += take
    while rem > 512 + 240:
        szs.append(512)
        rem -= 512
    if rem > 0:
        if rem <= 512:
            szs.append(rem)
        else:
            szs.append(rem - 272)
            szs.append(272)
    tiles = []
    off = 0
    for sz in szs:
        tiles.append((off, sz))
        off += sz
    return tiles


def _build(C):
    """Build the per-core Bass program for capacity C tokens."""
    nc = bacc.Bacc(None, target_bir_lowering=False)
    f16, f32 = mybir.dt.float16, mybir.dt.float32

    # ALL large inputs are host-packed per-partition-contiguous: the DMA
    # elem/packet size equals the min contiguous run of src/dst, and queues
    # moving small (512B) packets get starved ~6:1 by queues moving 4KB
    # packets, so every stream must present >=4KB runs per partition.
    #
    # xgp[p, 8*n0 + k*nsz + c] = x_gathered[token n0+c, k*128+p] for token
    # tile (n0, nsz).
    xgp = nc.dram_tensor("xgp", [128, KT_D * C], f16, kind="ExternalInput")
    # w1q[p, s, k*256+c]: SwiGLU pair chunk (slot s = W1_SLOT[i]) —
    # c<128 -> W1T[k*128+p, i*128+c] (h1), c>=128 -> W1T[k*128+p,
    # F+i*128+(c-128)] (h2).  Chunks are stored in queue-serving order
    # (gpsimd slots 0-8, sync slots 9-15) so each queue's stream can be
    # batched into few DMAs (fewer DMA semaphores -> shorter teardown).
    w1q = nc.dram_tensor("w1q", [128, KT_F, KT_D * 256], f16,
                         kind="ExternalInput")
    # w2q[p, kf, d] = W2T[kf*128+p, d]
    w2q = nc.dram_tensor("w2q", [128, KT_F, D], f16, kind="ExternalInput")
    # bc[:, :NF1] = b1 chunk columns, bc[:, NF1:] = b2 chunk columns
    bc = nc.dram_tensor("bc", [128, NF1 + NF2], f32, kind="ExternalInput")
    ytT = nc.dram_tensor("ytT", [D, C], f32, kind="ExternalOutput")

    Silu = mybir.ActivationFunctionType.Silu
    Ident = mybir.ActivationFunctionType.Identity

    tiles = _token_tiles(C)

    with tile.TileContext(nc) as tc:
        with (
            tc.tile_pool(name="wpool", bufs=1) as wpool,
            tc.tile_pool(name="apool", bufs=2) as apool,
            tc.tile_pool(name="tpool", bufs=4) as tpool,
            tc.tile_pool(name="opool", bufs=2) as opool,
            tc.tile_pool(name="psA", bufs=3, space="PSUM") as psA,
            tc.tile_pool(name="psB", bufs=2, space="PSUM") as psB,
        ):
            # Resident weights / biases / gathered activations.  xg_sb is
            # flat per-tile-block (same layout as xgp) so each tile's DMA is
            # a plain contiguous 2D copy -> 4KB packets, which win a fair
            # share of DMA-engine arbitration against the 4KB w1 stream.
            w1_sb = wpool.tile([128, KT_F, KT_D * 256], f16)
            w2_sb = wpool.tile([128, KT_F, D], f16)
            xg_sb = wpool.tile([128, KT_D * C], f16)
            bc_sb = wpool.tile([128, NF1 + NF2], f32)

            def xg_mv(n0, nsz, k):
                # moving operand [128, nsz] for contraction block k of the
                # token tile at (n0, nsz)
                base = KT_D * n0 + k * nsz
                return xg_sb[:, base:base + nsz]

            # Warm-up matmuls on a zeroed tile: keeps the PE busy during the
            # initial DMA wait so HAM unthrottles (1.2 -> 2.4 GHz) before
            # the first real matmul.  Each warmup is ~107ns (LDWEIGHTS-
            # bound); preamble ends ~7us, first tile's data lands ~13.5us
            # -> 64 covers ~6.8us.
            warm_sb = wpool.tile([128, 128], f16)
            nc.vector.memset(warm_sb, 0.0)
            warm_ps = psB.tile([128, NT], f32, tag="psb")
            for _ in range(64):
                nc.tensor.matmul(warm_ps[:, :128], warm_sb, warm_sb,
                                 start=True, stop=True)

            ytr = ytT.rearrange("(j p) c -> p j c", p=128)
            # DMA scheduling: the DMA engine pool drains the earliest-rung
            # queue ~exclusively before moving on, so the head's critical
            # path is SERIAL: xg tile 0 (sync queue, rings first), then w1
            # chunk 0 (gpsimd queue).  Everything else must stay out of the
            # way until w1 streaming is ahead: xg tiles 1+ are issued on
            # sync only after a dummy DMA whose dependency (the first
            # SwiGLU product) clears ~2us into the compute, and w2 rides at
            # the end of the gpsimd stream (first needed at mm2 of tile 0).
            # The biases go on the scalar queue, which then frees up for
            # ACT_TABLE_LOAD well before the first silu.
            # Weights ride TWO queues (gpsimd + sync): a single DMA queue
            # sustains only ~250-330 GB/s while the fabric can aggregate
            # ~450+, so alternating w1 chunks across both queues roughly
            # doubles weight-delivery rate.  The sync queue leads with xg
            # tile 0 (the first matmul group's moving operand); the later
            # xg tiles ride at the END of sync's weight stream — queue FIFO
            # order keeps them out of the head, and they still land long
            # before tile 1's matmuls need them.  The scalar engine only
            # issues the tiny bias DMA, keeping it free for the SwiGLU
            # ACTIVATEs that start ~2us into the compute.
            # The head's critical bytes (xg tile 0 + w1 chunk 0) are split
            # across both queues in proportion to their observed start times
            # (sync flows ~1.8us before gpsimd) so both finish together.
            n0, nsz = tiles[0]
            nc.sync.dma_start(out=xg_sb[:, :KT_D * nsz],
                              in_=xgp[:, :KT_D * nsz])
            nc.gpsimd.dma_start(out=w1_sb[:, 0, 0:1792], in_=w1q[:, 0, 0:1792])
            nc.scalar.dma_start(out=bc_sb, in_=bc[:, :])
            nc.sync.dma_start(out=w1_sb[:, 0, 1792:2048],
                              in_=w1q[:, 0, 1792:2048])
            # gpsimd: c1 alone (needed first), then pairs, then c15 + w2a.
            # sync (behind xg tile 0): pairs, then c14 + w2b + xg tiles 1+.
            nc.gpsimd.dma_start(out=w1_sb[:, 1, :], in_=w1q[:, 1, :])
            for s in (2, 4, 6):
                nc.gpsimd.dma_start(out=w1_sb[:, s:s + 2, :],
                                    in_=w1q[:, s:s + 2, :])
            nc.gpsimd.dma_start(out=w1_sb[:, 8, :], in_=w1q[:, 8, :])
            for s in (9, 11, 13):
                nc.sync.dma_start(out=w1_sb[:, s:s + 2, :],
                                  in_=w1q[:, s:s + 2, :])
            nc.sync.dma_start(out=w1_sb[:, 15, :], in_=w1q[:, 15, :])
            nc.gpsimd.dma_start(out=w2_sb[:, 0:8, :], in_=w2q[:, 0:8, :])
            nc.sync.dma_start(out=w2_sb[:, 8:16, :], in_=w2q[:, 8:16, :])
            for m0, msz in tiles[1:]:
                nc.sync.dma_start(
                    out=xg_sb[:, KT_D * m0:KT_D * (m0 + msz)],
                    in_=xgp[:, KT_D * m0:KT_D * (m0 + msz)])

            for ti, (n0, nsz) in enumerate(tiles):
                a_t = apool.tile([128, KT_F, NT], f16, tag="a")
                # ---- matmul 1 + SwiGLU: aT = (h1+b1a) * silu(h2+b1b)
                for i in range(KT_F):
                    ps1 = psA.tile([128, NT], f32, tag="ps1")
                    ps2 = psA.tile([128, NT], f32, tag="ps2")
                    si = W1_SLOT[i]
                    for k in range(KT_D):
                        nc.tensor.matmul(
                            ps1[:, :nsz],
                            w1_sb[:, si, k * 256:k * 256 + 128],
                            xg_mv(n0, nsz, k),
                            start=(k == 0),
                            stop=(k == KT_D - 1),
                        )
                    for k in range(KT_D):
                        nc.tensor.matmul(
                            ps2[:, :nsz],
                            w1_sb[:, si, k * 256 + 128:k * 256 + 256],
                            xg_mv(n0, nsz, k),
                            start=(k == 0),
                            stop=(k == KT_D - 1),
                        )
                    s_t = tpool.tile([128, NT], f32, tag="s")
                    nc.scalar.activation(
                        s_t[:, :nsz], ps2[:, :nsz], Silu,
                        bias=bc_sb[:, KT_F + i:KT_F + i + 1],
                    )
                    h_t = tpool.tile([128, NT], f32, tag="h")
                    nc.vector.tensor_scalar_add(
                        h_t[:, :nsz], ps1[:, :nsz], bc_sb[:, i:i + 1]
                    )
                    nc.vector.tensor_mul(
                        a_t[:, i, :nsz], h_t[:, :nsz], s_t[:, :nsz]
                    )

                # ---- matmul 2: yT = W2 @ aT + b2
                # j0-j3 outputs batched into one DMA (fewer DMA semaphores
                # -> shorter teardown epilogue); j4-j7 individual so the
                # post-last-matmul drain transfer stays small.
                o4_t = opool.tile([128, 4, NT], f32, tag="o4")
                for j in range(NF2):
                    ps = psB.tile([128, NT], f32, tag="psb")
                    for kf in range(KT_F):
                        nc.tensor.matmul(
                            ps[:, :nsz],
                            w2_sb[:, kf, j * 128:(j + 1) * 128],
                            a_t[:, kf, :nsz],
                            start=(kf == 0),
                            stop=(kf == KT_F - 1),
                        )
                    if j < 4:
                        nc.scalar.activation(
                            o4_t[:, j, :nsz], ps[:, :nsz], Ident,
                            bias=bc_sb[:, NF1 + j:NF1 + j + 1],
                        )
                        if j == 3:
                            nc.sync.dma_start(
                                out=ytr[:, 0:4, n0:n0 + nsz],
                                in_=o4_t[:, :, :nsz],
                            )
                    elif j < 6:
                        if j == 4:
                            o2_t = opool.tile([128, 2, NT], f32, tag="o2")
                        nc.scalar.activation(
                            o2_t[:, j - 4, :nsz], ps[:, :nsz], Ident,
                            bias=bc_sb[:, NF1 + j:NF1 + j + 1],
                        )
                        if j == 5:
                            nc.sync.dma_start(
                                out=ytr[:, 4:6, n0:n0 + nsz],
                                in_=o2_t[:, :, :nsz],
                            )
                    else:
                        o_t = opool.tile([128, NT], f32, tag="o")
                        nc.scalar.activation(
                            o_t[:, :nsz], ps[:, :nsz], Ident,
                            bias=bc_sb[:, NF1 + j:NF1 + j + 1],
                        )
                        nc.sync.dma_start(
                            out=ytr[:, j, n0:n0 + nsz],
                            in_=o_t[:, :nsz],
                        )
    nc.compile()
    return nc


def _get_nc(C):
    nc = _NC_CACHE.get(C)
    if nc is None:
        nc = _build(C)
        _NC_CACHE[C] = nc
    return nc


def _weights16(W1, W2):
    key = (W1.shape, W2.shape, W1.dtype.str, bytes(np.asarray(W1[0, 0, :8]).data),
           bytes(np.asarray(W2[0, 0, :8]).data))
    hit = _W_CACHE.get("w")
    if hit is not None and hit[0] == key:
        return hit[1], hit[2]
    # W1Q[e, p, i, k*256+c]: pair-packed W1 chunks (h1 half then h2 half of
    # SwiGLU chunk i), per-partition-contiguous (4KB/partition per chunk).
    W1T = np.transpose(W1, (0, 2, 1)).astype(np.float16)  # [E, D, 2F]
    W1r = W1T.reshape(E, KT_D, 128, 2, KT_F, 128)  # [e, k, p, half, i, c]
    W1Q = np.transpose(W1r, (0, 2, 4, 1, 3, 5)).reshape(
        E, 128, KT_F, KT_D * 256)
    W1Q = np.ascontiguousarray(W1Q[:, :, W1_ORDER, :])
    W2T = np.transpose(W2, (0, 2, 1)).astype(np.float16)  # [E, F, D]
    # W2Q[e, p, kf, d] = W2T[e, kf*128+p, d]
    W2Q = np.ascontiguousarray(
        np.transpose(W2T.reshape(E, KT_F, 128, D), (0, 2, 1, 3)))
    _W_CACHE["w"] = (key, W1Q, W2Q)
    return W1Q, W2Q


def kernel(x, Wr, temp, W1, b1, W2, b2):
    x = np.asarray(x)
    xf = np.ascontiguousarray(x.reshape(T, D), dtype=np.float32)

    # ---- host router (f64 for a stable top-k ordering)
    logits = xf.astype(np.float64) @ np.asarray(Wr).astype(np.float64).T
    logits /= np.float64(np.asarray(temp).reshape(-1)[0])
    top_idx = np.argsort(-logits, axis=1, kind="stable")[:, :K]  # [T, K]
    top_v = np.take_along_axis(logits, top_idx, axis=1)
    top_v -= top_v.max(axis=1, keepdims=True)
    ex = np.exp(top_v)
    gates = (ex / ex.sum(axis=1, keepdims=True)).astype(np.float32)  # [T, K]

    # ---- dispatch: per-expert token lists
    idx_e = []
    gate_e = []
    for e in range(E):
        rows, slot = np.where(top_idx == e)
        idx_e.append(rows)
        gate_e.append(gates[rows, slot])
    counts = np.array([len(r) for r in idx_e])
    C = max(256, int(-(-counts.max() // 16) * 16))

    nc = _get_nc(C)

    xf16 = xf.astype(np.float16)
    W1Q, W2Q = _weights16(np.asarray(W1), np.asarray(W2))
    b1a = np.asarray(b1, dtype=np.float32)  # [E, 2F]
    b2a = np.asarray(b2, dtype=np.float32)  # [E, D]

    tiles = _token_tiles(C)
    in_maps = []
    for e in range(E):
        xg = np.zeros((C, D), np.float16)
        xg[:counts[e]] = xf16[idx_e[e]]
        # pack per token tile: xgp[p, 8*n0 + k*nsz + c] = xg[n0+c, k*128+p]
        xgk = xg.reshape(C, KT_D, 128)  # [tok, k, p]
        xgp = np.empty((128, KT_D * C), np.float16)
        for n0, nsz in tiles:
            blk = np.transpose(xgk[n0:n0 + nsz], (2, 1, 0))  # [p, k, tok]
            xgp[:, KT_D * n0:KT_D * (n0 + nsz)] = blk.reshape(128, -1)
        bc = np.concatenate(
            [b1a[e].reshape(NF1, 128).T, b2a[e].reshape(NF2, 128).T], axis=1)
        in_maps.append({
            "xgp": xgp,
            "w1q": W1Q[e],
            "w2q": W2Q[e],
            "bc": np.ascontiguousarray(bc),
        })

    kwargs = {}
    if os.environ.get("KERNEL_TRACE"):
        kwargs = {"trace": True}
    try:
        res = run_bass_kernel_spmd(nc, in_maps, core_ids=list(range(E)), **kwargs)
    except ModuleNotFoundError:
        # trace path needs antenv.axon_hooks, absent on some images
        os.environ["BASS_NEVER_TRACE"] = "1"
        res = run_bass_kernel_spmd(nc, in_maps, core_ids=list(range(E)))
    global LAST_RESULT
    LAST_RESULT = res

    out = np.zeros((T, D), np.float32)
    for e in range(E):
        cnt = counts[e]
        if cnt == 0:
            continue
        y = res.results[e]["ytT"][:, :cnt].T  # [cnt, D]
        # top-2 expert choices are distinct, so rows are unique per expert
        out[idx_e[e]] += gate_e[e][:, None] * y
    return out.reshape(B, S, D)


LAST_RESULT = None

